# revision 37
# baseline (speedup 1.0000x reference)
"""FFT-Conv2d as direct valid cross-correlation on Trainium2 (Bass/Tile).

Math: the reference's rfft2/einsum/irfft2 pipeline is exactly a *valid*
2-D cross-correlation plus bias:

    out[b, d, i, j] = sum_{c,u,v} signal[b, c, i+u, j+v] * weight[d, c, u, v]
                      + bias[d]

with signal [16, 32, 256, 256], weight [32, 32, 31, 31] -> out [16, 32, 226, 226].

Device strategy (data-parallel, 2 batch images per NeuronCore x 8 cores):
  - Contraction dim (PE partition axis, K=128) packs 4 kernel rows x 32
    input channels.  The signal is replicated into SBUF 4x with row shifts
    of 0..3 so that one SBUF row slice provides all 128 contraction rows.
  - Output dim (PE partition axis of PSUM, M=128) packs 4 kernel-column
    subshifts s=0..3 x 32 output channels.  A column block vb covers
    kernel columns 4*vb+s; the s-shift is resolved after accumulation by
    a shifted 4-way add across PSUM partition blocks.
  - Per output-row-pair: 8 row-groups x 8 col-blocks = 64 matmuls of
    [128 x 128] @ [128 x (2*229)] accumulated in one PSUM bank, then a
    3-op vector epilogue (2 shifted adds + fused add+bias) and a DMA out.

Kernel weights/columns beyond 31 are zero-padded on the host; the signal
is zero-padded by one row/column in SBUF so the padded taps multiply
zeros (never uninitialized memory).
"""

import os
import sys

import numpy as np

for _p in ("/opt/trn_rl_repo",):
    if _p not in sys.path and os.path.isdir(_p):
        sys.path.insert(0, _p)

import concourse.bacc as bacc
import concourse.mybir as mybir
import concourse.tile as tile
from concourse.bass_utils import run_bass_kernel_spmd

# Problem constants (hardcoded per harness contract).
B, C, H, W = 16, 32, 256, 256
D, KH = 32, 31
TH = TW = 226
NCORES = 8
BPC = B // NCORES  # batches per core
HALO = 30          # extra sigrep rows below a tile (28 group offset + 2 wrap)
# Output-row tiles (start, nrows); nrows even.
ROW_TILES = [(0, 38), (38, 38), (76, 38), (114, 38), (152, 38), (190, 36)]

# key -> (weight dtype, signal dtype, use 3-D two-row rhs AP of width 229)
# float32r requires a 2-D (flat 512) moving AP; 16-bit dtypes can use the
# narrower 3-D AP (458 streamed columns instead of 512).
_DT_CONFIGS = {
    "f32r": (mybir.dt.float32r, mybir.dt.float32r, False),
    "f16": (mybir.dt.float16, mybir.dt.float16, True),
    "f16flat": (mybir.dt.float16, mybir.dt.float16, False),
    "bf16": (mybir.dt.bfloat16, mybir.dt.bfloat16, True),
    "f32": (mybir.dt.float32, mybir.dt.float32, True),
}
# f16 measured fastest on HW (one LDWEIGHTS per matmul is unavoidable with
# this toolchain; fp16 halves the weight-load and gets FWL).  rel err vs the
# fp32 FFT reference ~2.8e-4; use FFTCONV_DT=f32r for ~1.4e-4 at +15% time.
DT_KEY = os.environ.get("FFTCONV_DT", "f16")
# Row-pairs per weight-stationary wave (= PSUM banks cycled).  Measured on
# HW: WAVE=1 (bank-stable, weights reloaded per matmul) beats WAVE=8
# (stationary reuse but per-matmul PSUM bank switching stalls the PE).
WAVE = int(os.environ.get("FFTCONV_WAVE", "1"))


def _np_dt(dt_mm):
    return mybir.dt.np(dt_mm)


def build_program(dt_key: str = DT_KEY, repeat: int = 1):
    """Build the SPMD Bass program (one NeuronCore's slice: BPC batches)."""
    wt_dt, dt_mm, use3d = _DT_CONFIGS[dt_key]
    f32 = mybir.dt.float32
    NJ = 229
    # 0 = self-loading matmuls; 1 = explicit ldweights + ldweights=False
    # flags (needs walrus --enable-ldw-opt=false); 2 = ldweights=False flags
    # only (needs --enable-ldw-opt=true, which rejects explicit InstLdweights).
    # Standalone ldweights is rejected for 4-byte dtypes (fp32/fp32r).
    LDW_SHARE = (
        int(os.environ.get("FFTCONV_LDW_SHARE", "0"))
        if wt_dt in (mybir.dt.float16, mybir.dt.bfloat16)
        else 0
    )
    nc = bacc.Bacc(
        "TRN2",
        target_bir_lowering=False,
        debug=False,
        enable_asserts=False,
        num_devices=NCORES,
    )
    sig_d = nc.dram_tensor("signal", [BPC, C, H, W], dt_mm, kind="ExternalInput")
    wt_d = nc.dram_tensor("wT", [128, 8, 8, 128], wt_dt, kind="ExternalInput")
    bias_d = nc.dram_tensor("bias", [D, 1], f32, kind="ExternalInput")
    out_d = nc.dram_tensor("out", [BPC, D, TH, TW], f32, kind="ExternalOutput")

    SIG_BUFS = int(os.environ.get("FFTCONV_SIG_BUFS", "3"))
    TMP_BUFS = int(os.environ.get("FFTCONV_TMP_BUFS", "4"))
    OUT_BUFS = int(os.environ.get("FFTCONV_OUT_BUFS", "8"))
    with tile.TileContext(nc) as tc:
        with (
            tc.tile_pool(name="const", bufs=1) as const_pool,
            tc.tile_pool(name="sig", bufs=SIG_BUFS) as sig_pool,
            tc.tile_pool(name="psum", bufs=8, space="PSUM") as psum_pool,
            tc.tile_pool(name="tmp", bufs=TMP_BUFS) as tmp_pool,
            tc.tile_pool(name="outb", bufs=OUT_BUFS) as out_pool,
        ):
            wt = const_pool.tile([128, 8, 8, 128], wt_dt)
            nc.sync.dma_start(wt[:, :, :, :], wt_d[:, :, :, :])
            bias_t = const_pool.tile([D, 1], f32)
            nc.sync.dma_start(bias_t[:, :], bias_d[:, :])

            for b in [bb for _ in range(repeat) for bb in range(BPC)]:
                for i0, R in ROW_TILES:
                    # Signal rows stored CONTIGUOUSLY at pitch 256 (= W) so a
                    # two-row matmul rhs is one flat 512 span (float32r
                    # requires a 2-D moving AP).  Column overruns wrap to the
                    # next row but only land on zero-weight taps / unread
                    # psum columns.
                    r_tot = R + HALO
                    srep = sig_pool.tile([128, r_tot * W], dt_mm, tag="srep")
                    srep3 = srep[:].rearrange("p (r w) -> p r w", w=W)
                    for u in range(4):
                        rows = min(r_tot, H - (i0 + u))
                        nc.sync.dma_start(
                            srep3[u * 32 : (u + 1) * 32, 0:rows, :],
                            sig_d[b, :, i0 + u : i0 + u + rows, :],
                        )
                        if rows < r_tot:
                            nc.vector.memset(
                                srep3[u * 32 : (u + 1) * 32, rows:r_tot, :].bitcast(
                                    mybir.dt.float32
                                ),
                                0.0,
                            )
                    all_rps = list(range(R // 2))
                    for w0 in range(0, len(all_rps), WAVE):
                        wave = all_rps[w0 : w0 + WAVE]
                        # Weight-stationary: each (g, vb) lhsT streams all
                        # row-pairs of the wave (distinct PSUM banks) before
                        # the next weight load.
                        ps3s = []
                        for rp in wave:
                            if use3d:
                                ps_t = psum_pool.tile([128, 2, NJ], f32, tag="ps")
                                ps3s.append(ps_t)
                            else:
                                ps_t = psum_pool.tile([128, 2 * W], f32, tag="ps")
                                ps3s.append(ps_t[:].rearrange("p (r w) -> p r w", w=W))
                        for g in range(8):
                            for vb in range(8):
                                # One explicit weight load per (g, vb); the
                                # wave's matmuls reuse the stationary operand
                                # (ldweights=False skips the per-matmul load
                                # walrus would otherwise emit).
                                if LDW_SHARE == 1 and len(wave) > 1:
                                    nc.tensor.ldweights(wt[:, g, vb, :])
                                for j, rp in enumerate(wave):
                                    off = (2 * rp + 4 * g) * W + 4 * vb
                                    ps3 = ps3s[j]
                                    if use3d:
                                        rhs = srep[:, off : off + 2 * W].rearrange(
                                            "p (r w) -> p r w", w=W
                                        )[:, :, 0:NJ]
                                        out_ap = ps3[:, :, :]
                                    else:
                                        rhs = srep[:, off : off + 2 * W]
                                        out_ap = ps3.rearrange("p r w -> p (r w)")
                                    mm = nc.tensor.matmul(
                                        out_ap,
                                        lhsT=wt[:, g, vb, :],
                                        rhs=rhs,
                                        start=(g == 0 and vb == 0),
                                        stop=(g == 7 and vb == 7),
                                    )
                                    if LDW_SHARE and len(wave) > 1 and j > 0:
                                        mm.ins.ldweights = False
                                    elif LDW_SHARE == 1 and len(wave) > 1:
                                        # explicit ldweights above covers it
                                        mm.ins.ldweights = False
                        for j, rp in enumerate(wave):
                            i = i0 + 2 * rp
                            ps3 = ps3s[j]
                            # One PSUM operand per instruction (HW: single DVE
                            # PSUM read port).  ACT folds in the bias.
                            t0 = tmp_pool.tile([D, 2, TW], f32, tag="t0")
                            t1 = tmp_pool.tile([D, 2, TW], f32, tag="t1")
                            t2 = tmp_pool.tile([D, 2, TW], f32, tag="t2")
                            ob = out_pool.tile([D, 2, TW], f32, tag="ob")
                            nc.scalar.activation(
                                t0[:, :, :],
                                ps3[0:32, :, 0:226],
                                mybir.ActivationFunctionType.Identity,
                                bias=bias_t[:, :],
                            )
                            nc.vector.tensor_add(t1[:, :, :], t0[:, :, :], ps3[32:64, :, 1:227])
                            nc.vector.tensor_add(t2[:, :, :], t1[:, :, :], ps3[64:96, :, 2:228])
                            nc.vector.tensor_add(ob[:, :, :], t2[:, :, :], ps3[96:128, :, 3:229])
                            nc.sync.dma_start(out_d[b, :, i : i + 2, :], ob[:, :, :])
    nc.compile()
    # Off by default: only useful with WAVE>1 weight-stationary ordering,
    # which measured slower on HW (PSUM bank cycling).
    if int(os.environ.get("FFTCONV_LDW_DEDUP", "0")):
        bir = _dedupe_ldweights_json(nc.to_json_bytes())
        nc.to_json_bytes = lambda: bir  # instance override; cached bytes
    return nc


def _dedupe_ldweights_json(bir: bytes) -> bytes:
    """Drop PE Ldweights whose stationary operand is already loaded.

    tile_legalize splits every Matmult into Ldweights + Matmult(ldweights
    =false); with weight-stationary waves most loads are redundant reloads
    of the identical operand (measured ~107 ns each, serialized with the
    matmul stream).  Walrus's own dedupe (--enable-ldw-opt) is disabled in
    this toolchain, so do it on the serialized BIR: remove a Ldweights if
    the previous PE array load had the same operands/flags, carrying its
    semaphore waits/updates onto the next PE instruction.
    """
    import json as _json

    j = _json.loads(bir)
    removed = 0
    for fn in j.get("functions", []):
        for blk in fn.get("blocks", []):
            ins_l = blk.get("instructions")
            if not ins_l:
                continue
            out = []
            cur_sig = None
            for inst in ins_l:
                if inst.get("engine") != "PE":
                    out.append(inst)
                    continue
                op = inst.get("opcode")
                if op == "Ldweights":
                    sig = (
                        _json.dumps(inst.get("ins"), sort_keys=True),
                        inst.get("is_transpose"),
                        str(inst.get("perf_mode")),
                        str(inst.get("tile_position")),
                        str(inst.get("tile_size")),
                    )
                    si = inst.get("sync_info") or {}
                    if (
                        sig == cur_sig
                        and not (si.get("on_wait") or si.get("on_update"))
                    ):
                        # bare redundant reload: safe to drop (a Matmult can
                        # hold at most one ISA wait, so loads carrying sync
                        # stay).
                        removed += 1
                        continue
                    cur_sig = sig
                elif op == "Matmult":
                    if inst.get("ldweights") is not False:
                        cur_sig = None  # self-loading matmul replaces stationary
                elif op == "EventSemaphore":
                    pass  # pure semaphore op, array state unaffected
                else:
                    cur_sig = None  # Drain / branch: conservative reset
                out.append(inst)
            blk["instructions"] = out
    if removed:
        sys.stderr.write(f"[kernel] deduped {removed} redundant Ldweights\n")
    return _json.dumps(j).encode()


def pack_weights(weight: np.ndarray, np_dt) -> np.ndarray:
    """weight [D, C, 31, 31] -> lhsT table [128, 8, 8, 128].

    wT[(u_idx*32 + c), g, vb, (s*32 + d)] = weight[d, c, 4g+u_idx, 4vb+s],
    zero where 4g+u_idx > 30 or 4vb+s > 30.
    """
    w = np.zeros((D, C, 32, 32), np.float32)
    w[:, :, :KH, :KH] = weight.astype(np.float32)
    # -> [u_idx, c, g, vb, s, d]
    wt = w.reshape(D, C, 8, 4, 8, 4).transpose(3, 1, 2, 4, 5, 0)
    wt = wt.reshape(4 * C, 8, 8, 4 * D)
    return np.ascontiguousarray(wt.astype(np_dt))


_PROGRAM_CACHE: dict[str, object] = {}


def _get_program(dt_key: str):
    key = (dt_key, WAVE)
    prog = _PROGRAM_CACHE.get(key)
    if prog is None:
        prog = build_program(dt_key)
        _PROGRAM_CACHE[key] = prog
    return prog


def make_in_maps(signal, weight, bias, dt_key: str = DT_KEY):
    wt_dt, sig_dt, _ = _DT_CONFIGS[dt_key]
    wT = pack_weights(np.asarray(weight), _np_dt(wt_dt))
    sig = np.asarray(signal).astype(_np_dt(sig_dt), copy=False)
    b2 = np.ascontiguousarray(np.asarray(bias, np.float32).reshape(D, 1))
    return [
        {
            "signal": np.ascontiguousarray(sig[c * BPC : (c + 1) * BPC]),
            "wT": wT,
            "bias": b2,
        }
        for c in range(NCORES)
    ]


class _Executor:
    """Cached jitted shard_map executor (re-jitting per call costs ~7 s).

    Outputs are fully written by the kernel each run, so the previous
    call's output buffers are donated as the next call's NEFF output
    operands (no fresh zero upload per call).
    """

    def __init__(self, nc):
        import jax
        from concourse.bass2jax import (
            _bass_exec_p,
            install_neuronx_cc_hook,
            partition_id_tensor,
        )
        from jax.sharding import Mesh, NamedSharding, PartitionSpec

        try:
            from jax.experimental.shard_map import shard_map
        except ImportError:
            from jax import shard_map

        install_neuronx_cc_hook()
        self.jax = jax
        part_name = nc.partition_id_tensor.name if nc.partition_id_tensor else None
        in_names, out_names, out_avals = [], [], []
        for alloc in nc.m.functions[0].allocations:
            if not isinstance(alloc, mybir.MemoryLocationSet):
                continue
            name = alloc.memorylocations[0].name
            if alloc.kind == "ExternalInput":
                if name != part_name:
                    in_names.append(name)
            elif alloc.kind == "ExternalOutput":
                out_names.append(name)
                out_avals.append(
                    jax.core.ShapedArray(
                        tuple(alloc.tensor_shape), mybir.dt.np(alloc.dtype)
                    )
                )
        self.in_names, self.out_names, self.out_avals = in_names, out_names, out_avals
        n_params = len(in_names)
        all_in = list(in_names) + list(out_names)
        if part_name is not None:
            all_in.append(part_name)

        def _body(*args):
            operands = list(args)
            if part_name is not None:
                operands.append(partition_id_tensor())
            return tuple(
                _bass_exec_p.bind(
                    *operands,
                    out_avals=tuple(out_avals),
                    in_names=tuple(all_in),
                    out_names=tuple(out_names),
                    lowering_input_output_aliases=(),
                    sim_require_finite=True,
                    sim_require_nnan=True,
                    nc=nc,
                )
            )

        devices = jax.devices()[:NCORES]
        mesh = Mesh(np.asarray(devices), ("core",))
        n_outs = len(out_names)
        self.fn = jax.jit(
            shard_map(
                _body,
                mesh=mesh,
                in_specs=(PartitionSpec("core"),) * (n_params + n_outs),
                out_specs=(PartitionSpec("core"),) * n_outs,
                check_rep=False,
            ),
            donate_argnums=tuple(range(n_params, n_params + n_outs)),
        )
        self.in_sharding = NamedSharding(mesh, PartitionSpec("core"))
        self.prev_outs = None

    def run(self, in_maps):
        jax = self.jax
        concat_in = [
            np.concatenate([np.asarray(m[n]) for m in in_maps], axis=0)
            for n in self.in_names
        ]
        dev_in = jax.device_put(concat_in, [self.in_sharding] * len(concat_in))
        outs = self.prev_outs
        if outs is None:
            outs = [
                np.zeros((NCORES * a.shape[0], *a.shape[1:]), a.dtype)
                for a in self.out_avals
            ]
        outs = self.fn(*dev_in, *outs)
        jax.block_until_ready(outs)
        host = {n: np.asarray(o) for n, o in zip(self.out_names, outs)}
        self.prev_outs = list(outs)
        return host


_EXECUTOR_CACHE: dict = {}


def _get_executor():
    key = (DT_KEY, WAVE)
    ex = _EXECUTOR_CACHE.get(key)
    if ex is None:
        ex = _Executor(_get_program(DT_KEY))
        _EXECUTOR_CACHE[key] = ex
    return ex


def kernel(signal, weight, bias):
    in_maps = make_in_maps(signal, weight, bias, DT_KEY)
    try:
        host = _get_executor().run(in_maps)
        out_full = host["out"]
    except Exception:
        # Fallback: the stock (slower, re-jitting) execution path.
        nc = _get_program(DT_KEY)
        res = run_bass_kernel_spmd(nc, in_maps, list(range(NCORES)))
        out_full = np.concatenate(
            [res.results[c]["out"] for c in range(NCORES)], axis=0
        )
    out = out_full.reshape(B, D, TH, TW)
    return np.ascontiguousarray(out.astype(np.float32, copy=False))
